# revision 1
# baseline (speedup 1.0000x reference)
"""Binary 3D dilation (star/6-connected structuring element) on 8 TRN2 cores.

out = (conv3d(x, star_kernel, pad=1) > 0)  for x in {0,1}^(2,1,256,256,256)

Decomposition per output voxel:
    s = (x[d-1] + x[d] + x[d+1])          # Z: D-axis 3-sum (incl center)
      + (x[h-1] + x[h+1])                 # H shifts
      + (x[w-1] + x[w+1])                 # a: W shifts
    out = sign(s)            (s >= 0, integer-valued, so sign == (s > 0))

Sharding: core k -> batch k//4, D-quarter k%4. Each core receives a
66-plane slab (64 output planes + 1 halo plane each side, zero-padded at
the volume boundary), so no cross-core communication is needed.

Number formats: values are 0/1 (sums <= 7), so fp8e4m3 / bf16 are exact.
DRAM I/O is fp8 with a partition-major layout (multi-KB DMA descriptor
runs); SBUF compute is bf16 (DVE 2x mode); the DMA casts in flight.

H-INTERLEAVED partition layout: partition p, sub-column c in {0,1} holds
row h = 2p + c. The H-stencil then needs rows from the OTHER parity at
partitions p-1,p / p,p+1 -- two bidiagonal matmuls whose matrix edges
encode the h=0/255 boundaries exactly, so there is no chunk-boundary
halo at all:
  PE   : psum[c0] = B0^T @ x[d,c1]   (B0[k,m] = [k in {m-1,m}])
         psum[c1] = B1^T @ x[d,c0]   (B1[k,m] = [k in {m,m+1}])
         psum    += I^T @ g          (g = a + Z, merged on DVE)
                 (or += I^T @ a + I^T @ Z for PE/DVE balance)
  DVE  : a = x[w-1]+x[w+1]; Z = x[d-1]+x[d+1] then += x[d]; g = a+Z
         (all block-granular over PBLK planes)
  ACT  : out = Sign(psum), batched over PBLK planes
  DMA  : overlapped block loads / stores (SWDGE, fp8<->bf16 cast).
"""

import sys

import numpy as np

if "/opt/trn_rl_repo" not in sys.path:
    sys.path.insert(0, "/opt/trn_rl_repo")

B = 2
D_TOT = 256
H = 256
W = 256
N_CORES = 8
D_SHARDS = 4             # D split per batch entry
D_OUT = D_TOT // D_SHARDS          # 64 output planes per core
D_IN = D_OUT + 2                   # + halo plane each side
PBLK = 4                           # output planes per block
IBLK = PBLK + 2                    # input planes per block (overlapped)
WP = W + 2                         # host-padded row width (zero cols 0, 257)
MERGE_K = 2                        # planes per block with g=a+Z merged on DVE

# 6-connected "star" structuring element mask (D,H,W offsets from center)
_STAR = np.zeros((3, 3, 3), bool)
_STAR[1, 1, 1] = _STAR[0, 1, 1] = _STAR[2, 1, 1] = True
_STAR[1, 0, 1] = _STAR[1, 2, 1] = True
_STAR[1, 1, 0] = _STAR[1, 1, 2] = True

# extra kwargs for run_bass_kernel_spmd (test.py sets trace=True here)
RUN_KWARGS: dict = {}
LAST_RESULTS = None


def _b0() -> np.ndarray:
    m = np.zeros((128, 128), np.float32)
    i = np.arange(128)
    m[i, i] = 1.0
    m[i[:-1], i[:-1] + 1] = 1.0   # k = m-1
    return m


def _b1() -> np.ndarray:
    m = np.zeros((128, 128), np.float32)
    i = np.arange(128)
    m[i, i] = 1.0
    m[i[1:], i[1:] - 1] = 1.0     # k = m+1
    return m


def build_nc(d_out: int = D_OUT, merge_k: int = MERGE_K):
    """Build the per-core Bass program (identical on all cores)."""
    import concourse.bass as bass
    import concourse.mybir as mybir
    import concourse.tile as tile

    f32 = mybir.dt.float32
    bf16 = mybir.dt.bfloat16
    fp8 = mybir.dt.float8e4

    d_in = d_out + 2
    assert d_out % PBLK == 0
    # small blocks at the ends shorten the pipeline head (first compute
    # starts after a smaller load) and tail (shorter mm->sign->store drain)
    if d_out >= 3 * PBLK:
        # middle blocks alternate merge_k 1/2 to balance DVE vs PE
        blocks = [(2, 1), (2, 1)]
        blocks += [(PBLK, 1 + (i % 2)) for i in range((d_out - 8) // PBLK)]
        blocks += [(2, 1), (2, 1)]
    else:
        blocks = [(PBLK, merge_k)] * (d_out // PBLK)

    nc = bass.Bass()
    # partition-major DRAM layouts: [p, plane, c, w(padded)], h = 2p + c
    x = nc.declare_dram_parameter("x", [128, d_in, 2, WP], fp8, isOutput=False)
    b0_d = nc.declare_dram_parameter("b0", [128, 128], bf16, isOutput=False)
    b1_d = nc.declare_dram_parameter("b1", [128, 128], bf16, isOutput=False)
    eye_d = nc.declare_dram_parameter("eye", [128, 128], bf16, isOutput=False)
    y = nc.declare_dram_parameter("y", [128, d_out, 2, W], fp8, isOutput=True)

    with tile.TileContext(nc) as tc:
        with (
            tc.tile_pool(name="consts", bufs=1) as cpool,
            tc.tile_pool(name="inblk", bufs=8) as ipool,
            tc.tile_pool(name="outblk", bufs=4) as opool,
            tc.tile_pool(name="awork", bufs=6) as apool,
            tc.tile_pool(name="psum", bufs=2, space=bass.MemorySpace.PSUM) as ppool,
        ):
            b0 = cpool.tile([128, 128], bf16, tag="b0")
            b1 = cpool.tile([128, 128], bf16, tag="b1")
            eye = cpool.tile([128, 128], bf16, tag="eye")
            nc.sync.dma_start(out=b0[:], in_=b0_d[:])
            nc.sync.dma_start(out=b1[:], in_=b1_d[:])
            nc.sync.dma_start(out=eye[:], in_=eye_d[:])

            p0 = 0
            for npl, mk in blocks:
                # ---- load (fp8 DRAM -> bf16 SBUF, cast in DMA) ------------
                blk = ipool.tile([128, npl + 2, 2, WP], bf16, tag="in")
                nc.gpsimd.dma_start(out=blk[:], in_=x[:, p0 : p0 + npl + 2])
                # ---- block-granular elementwise (DVE) ---------------------
                # aw[:, p, c, 0] = a = x[d,w-1] + x[d,w+1]
                # aw[:, p, c, 1] = Z = x[d-1] + x[d] + x[d+1]
                aw = apool.tile([128, npl, 2, 2, W], bf16, tag="aw")
                cur = blk[:, 1 : 1 + npl]
                av = aw[:, :, :, 0]
                zv = aw[:, :, :, 1]
                nc.vector.tensor_add(
                    out=zv,
                    in0=blk[:, 0:npl, :, 1 : W + 1],
                    in1=blk[:, 2 : 2 + npl, :, 1 : W + 1],
                )
                nc.vector.tensor_add(
                    out=zv, in0=zv, in1=cur[:, :, :, 1 : W + 1]
                )
                nc.vector.tensor_add(
                    out=av, in0=cur[:, :, :, 0:W], in1=cur[:, :, :, 2 : 2 + W]
                )
                if mk:
                    # g = a + Z for the LAST mk planes (into j=0) -- the
                    # unmerged planes' matmuls don't wait on this, so PE
                    # starts while the merge finishes
                    nc.vector.tensor_add(
                        out=aw[:, npl - mk :, :, 0],
                        in0=aw[:, npl - mk :, :, 0],
                        in1=aw[:, npl - mk :, :, 1],
                    )
                # ---- matmuls + one batched sign ---------------------------
                ot = opool.tile([128, npl, 2, W], fp8, tag="out")
                ps = ppool.tile([128, npl, 2 * W], f32, tag="ps")
                for dd in range(npl):
                    merged = dd >= npl - mk
                    # full-span matmul first: start=True zeroes the whole
                    # bank, later matmuls accumulate. For unmerged planes
                    # lead with eye@Z (ready earliest in the DVE chain).
                    nc.tensor.matmul(
                        ps[:, dd],
                        eye[:],
                        aw[:, dd, :, 1 - int(merged)],
                        start=True,
                        stop=False,
                        skip_group_check=True,
                    )
                    nc.tensor.matmul(
                        ps[:, dd, 0:W],
                        b0[:],
                        cur[:, dd, 1, 1 : W + 1],
                        start=False,
                        stop=False,
                        skip_group_check=True,
                    )
                    nc.tensor.matmul(
                        ps[:, dd, W : 2 * W],
                        b1[:],
                        cur[:, dd, 0, 1 : W + 1],
                        start=False,
                        stop=merged,
                        skip_group_check=True,
                    )
                    if not merged:
                        nc.tensor.matmul(
                            ps[:, dd],
                            eye[:],
                            aw[:, dd, :, 0],
                            start=False,
                            stop=True,
                            skip_group_check=True,
                        )
                nc.scalar.sign(
                    out=ot[:].rearrange("h p c w -> h (p c w)"),
                    in_=ps[:].rearrange("h p n -> h (p n)"),
                )
                # ---- store (fp8 SBUF -> fp8 DRAM, HWDGE) ------------------
                nc.sync.dma_start(out=y[:, p0 : p0 + npl], in_=ot[:])
                p0 += npl

    # Walrus codegen allows at most 1 semaphore wait per engine instruction
    # (2 on InstEventSemaphore). Run the bacc passes that enforce this.
    import bass_rust as _bass_rust

    _bass_rust.move_matmul_waits_to_ldweights(nc.m)
    _bass_rust.generate_event_semaphores(nc)
    return nc


_NC_CACHE = None


def host_inputs(slab_f32: np.ndarray) -> dict:
    """Per-core in_map from a zero-padded (d_in, H, WP) slab (0/1 values)."""
    import ml_dtypes

    f8 = ml_dtypes.float8_e4m3fn
    d_in = slab_f32.shape[0]
    # [plane, (p c), w] -> [p, plane, c, w]   (h = 2p + c interleave)
    xh = np.ascontiguousarray(
        slab_f32.reshape(d_in, 128, 2, WP).transpose(1, 0, 2, 3)
    ).astype(f8)
    return {
        "x": xh,
        "b0": _b0().astype(ml_dtypes.bfloat16),
        "b1": _b1().astype(ml_dtypes.bfloat16),
        "eye": np.eye(128, dtype=ml_dtypes.bfloat16),
    }


def out_to_slab(yh: np.ndarray) -> np.ndarray:
    """[p, d, c, w] fp8 -> (d, H, W) float32 (h = 2p + c)."""
    d_out = yh.shape[1]
    return (
        yh.astype(np.float32).transpose(1, 0, 2, 3).reshape(d_out, H, W)
    )


def _np_dilate(vol: np.ndarray, ker: np.ndarray) -> np.ndarray:
    """Generic numpy fallback: conv3d(pad=1) > 0 for an arbitrary 3x3x3
    kernel (matches the reference exactly, including negative weights)."""
    b, ch, dd, hh, ww = vol.shape
    pad = np.pad(vol, ((0, 0), (0, 0), (1, 1), (1, 1), (1, 1)))
    kv = ker.reshape(3, 3, 3).astype(np.float64)
    s = np.zeros(vol.shape, np.float64)
    for i in range(3):
        for j in range(3):
            for k in range(3):
                if kv[i, j, k] != 0.0:
                    s += kv[i, j, k] * pad[:, :, i : i + dd, j : j + hh, k : k + ww]
    return (s > 0).astype(vol.dtype)


def kernel(binary_volume=None, kernel=None, **_unused):
    global _NC_CACHE, LAST_RESULTS
    vol = np.ascontiguousarray(np.asarray(binary_volume), dtype=np.float32)
    ker = np.asarray(kernel, dtype=np.float32)
    kv = ker.reshape(3, 3, 3)
    if (
        vol.shape != (B, 1, D_TOT, H, W)
        or not np.array_equal(kv != 0, _STAR)
        or not (kv[_STAR] > 0).all()
        or not ((vol == 0.0) | (vol == 1.0)).all()
    ):
        return _np_dilate(vol, ker).astype(np.asarray(binary_volume).dtype)

    from concourse.bass_utils import run_bass_kernel_spmd

    xr = vol.reshape(B, D_TOT, H, W)
    in_maps = []
    for core in range(N_CORES):
        b, s = divmod(core, D_SHARDS)
        d0 = s * D_OUT
        slab = np.zeros((D_IN, H, WP), np.float32)
        j_lo = 0 if d0 > 0 else 1                      # slab j <-> global d0-1+j
        j_hi = D_IN if d0 + D_OUT < D_TOT else D_IN - 1
        slab[j_lo:j_hi, :, 1 : W + 1] = xr[b, d0 - 1 + j_lo : d0 - 1 + j_hi]
        in_maps.append(host_inputs(slab))

    if _NC_CACHE is None:
        _NC_CACHE = build_nc()
    res = run_bass_kernel_spmd(_NC_CACHE, in_maps, list(range(N_CORES)), **RUN_KWARGS)
    LAST_RESULTS = res

    full = np.empty((B, 1, D_TOT, H, W), np.float32)
    for core in range(N_CORES):
        b, s = divmod(core, D_SHARDS)
        full[b, 0, s * D_OUT : (s + 1) * D_OUT] = out_to_slab(
            res.results[core]["y"]
        )
    return full



# revision 4
# speedup vs baseline: 3.1132x; 3.1132x over previous
"""Binary 3D dilation (star/6-connected structuring element) on 8 TRN2 cores.

out = (conv3d(x, star_kernel, pad=1) > 0)  for x in {0,1}^(2,1,256,256,256)

Since the volume is 0/1, dilation is a pure bitwise OR of 7 shifted copies:

    out[d,h,w] = x[d-1] | x[d+1] | x[d,h-1] | x[d,h+1]
               | x[d,w-1] | x[d,w+1] | x[d]

BIT-PACKED formulation: the host packs 32 voxels (along W) per uint32
(little-endian: bit k of elem e = voxel w = 32e+k).  That is a pure
format cast (like fp32->fp8) but shrinks HBM traffic 8x and lets one
DVE lane process 32 voxels per cycle.  The W-stencil becomes in-element
shifts plus a cross-element carry, which scalar_tensor_tensor fuses
with the OR accumulate:  acc = (v << 1) | acc  etc.  Restricting the
carry ops to row-interior elements implements the w=0/255 boundary
exactly, so rows need NO guard padding.

Layout per core: partition p holds 4 overlapped rows 2p-1..2p+2
(c = 0..3), so every H-stencil term is a same-partition c-slice and the
D-terms are plane-offset views -- no cross-partition traffic at all.
Output rows per partition: 2p, 2p+1 (c = 1, 2 of the input layout).

Sharding: core k -> batch k//4, D-quarter k%4; each core gets a
66-plane slab (64 output planes + zero-padded halo plane each side).

Engine split per chunk of n planes (all ops full-chunk single shot):
  DVE  : acc  = (v<<1)|v ; acc = (v>>1)|acc          (W +- 1, in-elem)
         acc[e+1] |= v[e]>>31 ; acc[e-1] |= v[e]<<31 (W carry, in-row)
         acc |= pacc                                  (merge)
  Pool : pacc = x[d-1]|x[d+1] ; pacc |= x[h-1 rows] ; pacc |= x[h+1 rows]
  DMA  : sync HWDGE loads, scalar HWDGE stores (queues stripe over all
         16 DMA engines; uint32 both sides, no cast).
"""

import sys

import numpy as np

if "/opt/trn_rl_repo" not in sys.path:
    sys.path.insert(0, "/opt/trn_rl_repo")

B = 2
D_TOT = 256
H = 256
W = 256
WE = W // 32                       # uint32 elems per row
N_CORES = 8
D_SHARDS = 4                       # D split per batch entry
D_OUT = D_TOT // D_SHARDS          # 64 output planes per core
D_IN = D_OUT + 2                   # + halo plane each side
N_CHUNKS = 2                       # compute/store chunks per core

# 6-connected "star" structuring element mask (D,H,W offsets from center)
_STAR = np.zeros((3, 3, 3), bool)
_STAR[1, 1, 1] = _STAR[0, 1, 1] = _STAR[2, 1, 1] = True
_STAR[1, 0, 1] = _STAR[1, 2, 1] = True
_STAR[1, 1, 0] = _STAR[1, 1, 2] = True

# extra kwargs for run_bass_kernel_spmd (test.py sets trace=True here)
RUN_KWARGS: dict = {}
LAST_RESULTS = None


def build_nc(d_out: int = D_OUT, n_chunks: int = N_CHUNKS):
    """Build the per-core Bass program (identical on all cores)."""
    import concourse.bass as bass
    import concourse.mybir as mybir
    import concourse.tile as tile

    u32 = mybir.dt.uint32
    OR = mybir.AluOpType.bitwise_or
    SHL = mybir.AluOpType.logical_shift_left
    SHR = mybir.AluOpType.logical_shift_right

    d_in = d_out + 2
    assert d_out % n_chunks == 0
    n = d_out // n_chunks

    nc = bass.Bass()
    # x: [p, plane, c, we] with c = row 2p-1+c (4-row overlap); y: rows 2p, 2p+1
    x = nc.declare_dram_parameter("x", [128, d_in, 4, WE], u32, isOutput=False)
    y = nc.declare_dram_parameter("y", [128, d_out, 2, WE], u32, isOutput=True)

    with tile.TileContext(nc) as tc:
        with (
            tc.tile_pool(name="consts", bufs=1) as cpool,
            tc.tile_pool(name="xin", bufs=2) as xpool,
            tc.tile_pool(name="accp", bufs=2) as apool,
            tc.tile_pool(name="paccp", bufs=2) as ppool,
        ):
            # shift amounts as SBUF per-partition scalars (immediates are
            # lowered as fp32 -- unsafe as HW shift operands)
            c1 = cpool.tile([128, 1], u32, tag="c1")
            c31 = cpool.tile([128, 1], u32, tag="c31")
            nc.vector.memset(c1[:], 1)
            nc.vector.memset(c31[:], 31)

            for k in range(n_chunks):
                j0 = k * n
                xt = xpool.tile([128, n + 2, 4, WE], u32, tag="x")
                nc.sync.dma_start(out=xt[:], in_=x[:, j0 : j0 + n + 2])

                acc = apool.tile([128, n, 2, WE], u32, tag="acc")
                pc = ppool.tile([128, n, 2, WE], u32, tag="pacc")
                v = xt[:, 1 : n + 1, 1:3]          # center planes, out rows
                # stt is limited to <=3D APs: merged (c w) views for the
                # full-row ops, per-row-half views for the carry ops
                vv = v.rearrange("p j c w -> p j (c w)")
                av = acc[:].rearrange("p j c w -> p j (c w)")

                # ---- DVE: center + W-stencil (bit shifts + row carries) ----
                nc.vector.scalar_tensor_tensor(
                    out=av, in0=vv, scalar=c1[:], in1=vv, op0=SHL, op1=OR
                )
                nc.vector.scalar_tensor_tensor(
                    out=av, in0=vv, scalar=c1[:], in1=av, op0=SHR, op1=OR
                )
                for ci in (0, 1):
                    nc.vector.scalar_tensor_tensor(
                        out=acc[:, :, ci, 1:WE],
                        in0=xt[:, 1 : n + 1, 1 + ci, 0 : WE - 1],
                        scalar=c31[:],
                        in1=acc[:, :, ci, 1:WE],
                        op0=SHR,
                        op1=OR,
                    )
                for ci in (0, 1):
                    nc.vector.scalar_tensor_tensor(
                        out=acc[:, :, ci, 0 : WE - 1],
                        in0=xt[:, 1 : n + 1, 1 + ci, 1:WE],
                        scalar=c31[:],
                        in1=acc[:, :, ci, 0 : WE - 1],
                        op0=SHL,
                        op1=OR,
                    )
                # ---- DVE: D-stencil + H-stencil (aligned ORs; bitwise ops
                # on 32-bit ints are DVE-only per the walrus verifier) ------
                nc.vector.tensor_tensor(
                    out=pc[:], in0=xt[:, 0:n, 1:3], in1=xt[:, 2 : n + 2, 1:3], op=OR
                )
                nc.vector.tensor_tensor(out=acc[:], in0=pc[:], in1=acc[:], op=OR)
                # h-window: out row 2p needs rows 2p-1,2p+1 = c0|c2; row
                # 2p+1 needs 2p,2p+2 = c1|c3 -> one op over adjacent c-pairs
                nc.vector.tensor_tensor(
                    out=pc[:], in0=xt[:, 1 : n + 1, 0:2], in1=xt[:, 1 : n + 1, 2:4], op=OR
                )
                nc.vector.tensor_tensor(out=acc[:], in0=pc[:], in1=acc[:], op=OR)
                nc.scalar.dma_start(out=y[:, j0 : j0 + n], in_=acc[:])

    # Walrus codegen allows at most 1 semaphore wait per engine instruction.
    import bass_rust as _bass_rust

    _bass_rust.move_matmul_waits_to_ldweights(nc.m)
    _bass_rust.generate_event_semaphores(nc)
    return nc


_NC_CACHE = None


def _pack_bits(a: np.ndarray) -> np.ndarray:
    """(..., W) 0/1 float -> (..., WE) uint32, bit k of elem e = w = 32e+k."""
    b = np.packbits(a.astype(bool), axis=-1, bitorder="little")
    return np.ascontiguousarray(b).view("<u4")


def host_inputs(slab_f32: np.ndarray) -> dict:
    """Per-core in_map from a zero-padded (d_in, H, W) slab (0/1 values)."""
    d_in = slab_f32.shape[0]
    packed = _pack_bits(slab_f32)                     # (d_in, H, WE)
    P = np.zeros((d_in, H + 2, WE), np.uint32)
    P[:, 1 : H + 1] = packed
    # SW[j, r, w, t] = P[j, r+t, w]; row 2p+c of P = global row 2p-1+c
    SW = np.lib.stride_tricks.sliding_window_view(P, 4, axis=1)
    xh = np.ascontiguousarray(SW[:, 0::2].transpose(1, 0, 3, 2))
    return {"x": xh}                                   # (128, d_in, 4, WE)


def out_to_slab(yh: np.ndarray) -> np.ndarray:
    """[p, d, c, we] uint32 -> (d_out, H, W) float32 (h = 2p + c)."""
    d_out = yh.shape[1]
    rows = np.ascontiguousarray(yh.transpose(1, 0, 2, 3)).reshape(d_out, H, WE)
    bits = np.unpackbits(rows.view(np.uint8), axis=-1, bitorder="little")
    return bits.reshape(d_out, H, W).astype(np.float32)


def _np_dilate(vol: np.ndarray, ker: np.ndarray) -> np.ndarray:
    """Generic numpy fallback: conv3d(pad=1) > 0 for an arbitrary 3x3x3
    kernel (matches the reference exactly, including negative weights)."""
    b, ch, dd, hh, ww = vol.shape
    pad = np.pad(vol, ((0, 0), (0, 0), (1, 1), (1, 1), (1, 1)))
    kv = ker.reshape(3, 3, 3).astype(np.float64)
    s = np.zeros(vol.shape, np.float64)
    for i in range(3):
        for j in range(3):
            for k in range(3):
                if kv[i, j, k] != 0.0:
                    s += kv[i, j, k] * pad[:, :, i : i + dd, j : j + hh, k : k + ww]
    return (s > 0).astype(vol.dtype)


def kernel(binary_volume=None, kernel=None, **_unused):
    global _NC_CACHE, LAST_RESULTS
    vol = np.ascontiguousarray(np.asarray(binary_volume), dtype=np.float32)
    ker = np.asarray(kernel, dtype=np.float32)
    kv = ker.reshape(3, 3, 3)
    if (
        vol.shape != (B, 1, D_TOT, H, W)
        or not np.array_equal(kv != 0, _STAR)
        or not (kv[_STAR] > 0).all()
        or not ((vol == 0.0) | (vol == 1.0)).all()
    ):
        return _np_dilate(vol, ker).astype(np.asarray(binary_volume).dtype)

    from concourse.bass_utils import run_bass_kernel_spmd

    xr = vol.reshape(B, D_TOT, H, W)
    in_maps = []
    for core in range(N_CORES):
        b, s = divmod(core, D_SHARDS)
        d0 = s * D_OUT
        slab = np.zeros((D_IN, H, W), np.float32)
        j_lo = 0 if d0 > 0 else 1                      # slab j <-> global d0-1+j
        j_hi = D_IN if d0 + D_OUT < D_TOT else D_IN - 1
        slab[j_lo:j_hi] = xr[b, d0 - 1 + j_lo : d0 - 1 + j_hi]
        in_maps.append(host_inputs(slab))

    if _NC_CACHE is None:
        _NC_CACHE = build_nc()
    res = run_bass_kernel_spmd(_NC_CACHE, in_maps, list(range(N_CORES)), **RUN_KWARGS)
    LAST_RESULTS = res

    full = np.empty((B, 1, D_TOT, H, W), np.float32)
    for core in range(N_CORES):
        b, s = divmod(core, D_SHARDS)
        full[b, 0, s * D_OUT : (s + 1) * D_OUT] = out_to_slab(
            res.results[core]["y"]
        )
    return full


# revision 5
# speedup vs baseline: 3.9632x; 1.2730x over previous
"""Binary 3D dilation (star/6-connected structuring element) on 8 TRN2 cores.

out = (conv3d(x, star_kernel, pad=1) > 0)  for x in {0,1}^(2,1,256,256,256)

Since the volume is 0/1, dilation is a pure bitwise OR of 7 shifted copies:

    out[d,h,w] = x[d-1] | x[d+1] | x[d,h-1] | x[d,h+1]
               | x[d,w-1] | x[d,w+1] | x[d,w]

BIT-PACKED formulation (host-side pure format cast, like fp32->fp8, but
8x smaller): 30 fresh voxels per uint32 with a 1-bit halo each side --
elem e of a row holds voxels 30e-1 .. 30e+30 in bits 0..31 (little
endian; valid output bits are 1..30).  The in-element halo makes the
W-stencil SELF-CONTAINED per element:  (v<<1)|v|(v>>1)  needs no
cross-element carry, so no guard elements and no boundary fixups; the
host discards bits 0/31 on unpack.  A 256-voxel row is ceil(256/30)=9
elems = 36B.

Partition layout: partition p holds 4 overlapped rows 2p-1..2p+2
(c = 0..3), so every H-stencil term is a same-partition c-slice and the
D-terms are plane-offset views -- no cross-partition traffic.  The
H-window collapses to ONE op:  out rows (2p, 2p+1) need (c0|c2, c1|c3)
= x[c0:2] | x[c2:4].  Output rows per partition: 2p, 2p+1.

Per chunk of n planes the whole dilation is SIX DVE instructions
(bitwise ops are DVE-only on TRN2; Pool/ACT were probed and rejected by
the walrus verifier/codegen):
    acc = (v<<1)|v ; acc = (v>>1)|acc          [scalar_tensor_tensor]
    pc = x[d-1]|x[d+1] ; acc |= pc             [tensor_tensor]
    pc2 = x[c0:2]|x[c2:4] ; acc |= pc2         [tensor_tensor]
The final merge+store of the LAST chunk is split in half so the last
store packet lands right after the last DVE op.

Sharding: core k -> batch k//4, D-quarter k%4; each core gets a
66-plane slab (64 output planes + zero-padded halo plane each side).
DMA: loads split across the Sync+Scalar HWDGE queues (each stripes over
all 16 DMA engines), stores on the opposite queue per chunk.
"""

import sys

import numpy as np

if "/opt/trn_rl_repo" not in sys.path:
    sys.path.insert(0, "/opt/trn_rl_repo")

B = 2
D_TOT = 256
H = 256
W = 256
VPE = 30                           # fresh voxels per uint32 elem
WE = -(-W // VPE)                  # 9 elems per 256-voxel row
N_CORES = 8
D_SHARDS = 4                       # D split per batch entry
D_OUT = D_TOT // D_SHARDS          # 64 output planes per core
D_IN = D_OUT + 2                   # + halo plane each side
N_CHUNKS = 2                       # compute chunks per core

# 6-connected "star" structuring element mask (D,H,W offsets from center)
_STAR = np.zeros((3, 3, 3), bool)
_STAR[1, 1, 1] = _STAR[0, 1, 1] = _STAR[2, 1, 1] = True
_STAR[1, 0, 1] = _STAR[1, 2, 1] = True
_STAR[1, 1, 0] = _STAR[1, 1, 2] = True

# extra kwargs for run_bass_kernel_spmd (test.py sets trace=True here)
RUN_KWARGS: dict = {}
LAST_RESULTS = None


def build_nc(d_out: int = D_OUT, n_chunks: int = N_CHUNKS):
    """Build the per-core Bass program (identical on all cores)."""
    import concourse.bass as bass
    import concourse.mybir as mybir
    import concourse.tile as tile

    u32 = mybir.dt.uint32
    OR = mybir.AluOpType.bitwise_or
    SHL = mybir.AluOpType.logical_shift_left
    SHR = mybir.AluOpType.logical_shift_right

    d_in = d_out + 2
    assert d_out % n_chunks == 0
    n = d_out // n_chunks

    nc = bass.Bass()
    # x: [p, plane, c, we] with c = row 2p-1+c (4-row overlap); y: rows 2p, 2p+1
    x = nc.declare_dram_parameter("x", [128, d_in, 4, WE], u32, isOutput=False)
    y = nc.declare_dram_parameter("y", [128, d_out, 2, WE], u32, isOutput=True)

    with tile.TileContext(nc) as tc:
        with (
            tc.tile_pool(name="consts", bufs=1) as cpool,
            tc.tile_pool(name="xin", bufs=2) as xpool,
            tc.tile_pool(name="accp", bufs=2) as apool,
            tc.tile_pool(name="paccp", bufs=2) as ppool,
        ):
            # shift amount as an SBUF per-partition scalar (immediates are
            # lowered as fp32 -- unsafe as HW shift operands)
            c1 = cpool.tile([128, 1], u32, tag="c1")
            nc.vector.memset(c1[:], 1)

            for k in range(n_chunks):
                j0 = k * n
                xt = xpool.tile([128, n + 2, 4, WE], u32, tag="x")
                # split each chunk load across both HWDGE queues
                h = (n + 2) // 2
                nc.sync.dma_start(out=xt[:, 0:h], in_=x[:, j0 : j0 + h])
                nc.scalar.dma_start(
                    out=xt[:, h : n + 2], in_=x[:, j0 + h : j0 + n + 2]
                )

                acc = apool.tile([128, n, 2, WE], u32, tag="acc")
                pc = ppool.tile([128, n, 2, WE], u32, tag="pc")
                v = xt[:, 1 : n + 1, 1:3]          # center planes, out rows
                vv = v.rearrange("p j c w -> p j (c w)")
                av = acc[:].rearrange("p j c w -> p j (c w)")

                # ---- W-stencil: self-contained in-element shifts ----------
                nc.vector.scalar_tensor_tensor(
                    out=av, in0=vv, scalar=c1[:], in1=vv, op0=SHL, op1=OR
                )
                nc.vector.scalar_tensor_tensor(
                    out=av, in0=vv, scalar=c1[:], in1=av, op0=SHR, op1=OR
                )
                # ---- D-stencil pair + merge -------------------------------
                nc.vector.tensor_tensor(
                    out=pc[:], in0=xt[:, 0:n, 1:3], in1=xt[:, 2 : n + 2, 1:3], op=OR
                )
                nc.vector.tensor_tensor(out=acc[:], in0=pc[:], in1=acc[:], op=OR)
                # ---- H-window pair + merge (+ store) ----------------------
                nc.vector.tensor_tensor(
                    out=pc[:], in0=xt[:, 1 : n + 1, 0:2], in1=xt[:, 1 : n + 1, 2:4],
                    op=OR,
                )
                last = k == n_chunks - 1
                # final chunk: split the last merge so the first half's
                # store flies while the second half computes
                nsplit = 2 if last else 1
                m = n // nsplit
                for s in range(nsplit):
                    sl = slice(s * m, (s + 1) * m)
                    nc.vector.tensor_tensor(
                        out=acc[:, sl], in0=pc[:, sl], in1=acc[:, sl], op=OR
                    )
                    eng = nc.scalar if (k + s) % 2 == 0 else nc.sync
                    eng.dma_start(
                        out=y[:, j0 + s * m : j0 + (s + 1) * m], in_=acc[:, sl]
                    )

    # Walrus codegen allows at most 1 semaphore wait per engine instruction.
    import bass_rust as _bass_rust

    _bass_rust.move_matmul_waits_to_ldweights(nc.m)
    _bass_rust.generate_event_semaphores(nc)
    return nc


_NC_CACHE = None


def _pack_bits(a: np.ndarray) -> np.ndarray:
    """(..., W) 0/1 -> (..., WE) uint32; elem e bit b = voxel 30e-1+b."""
    lead = a.shape[:-1]
    w = a.shape[-1]
    xp = np.zeros(lead + (VPE * (WE - 1) + 33,), bool)
    xp[..., 1 : w + 1] = a != 0
    win = np.lib.stride_tricks.sliding_window_view(xp, 32, axis=-1)[..., ::VPE, :]
    b = np.packbits(np.ascontiguousarray(win), axis=-1, bitorder="little")
    return b.reshape(lead + (WE * 4,)).view("<u4")


def _unpack_bits(p: np.ndarray) -> np.ndarray:
    """(..., WE) uint32 -> (..., W) float32 (valid bits 1..30 per elem)."""
    lead = p.shape[:-1]
    u8 = np.ascontiguousarray(p).view(np.uint8).reshape(lead + (WE, 4))
    bits = np.unpackbits(u8, axis=-1, bitorder="little").reshape(lead + (WE, 32))
    return (
        bits[..., 1:31].reshape(lead + (WE * VPE,))[..., :W].astype(np.float32)
    )


def host_inputs(slab_f32: np.ndarray) -> dict:
    """Per-core in_map from a D-zero-padded (d_in, H, W) slab (0/1 values)."""
    d_in = slab_f32.shape[0]
    packed = _pack_bits(slab_f32)                     # (d_in, H, WE)
    P = np.zeros((d_in, H + 2, WE), np.uint32)
    P[:, 1 : H + 1] = packed
    # SW[j, r, w, t] = P[j, r+t, w]; row 2p+c of P = global row 2p-1+c
    SW = np.lib.stride_tricks.sliding_window_view(P, 4, axis=1)
    xh = np.ascontiguousarray(SW[:, 0::2].transpose(1, 0, 3, 2))
    return {"x": xh}                                   # (128, d_in, 4, WE)


def out_to_slab(yh: np.ndarray) -> np.ndarray:
    """[p, d, c, we] uint32 -> (d_out, H, W) float32 (h = 2p + c)."""
    d_out = yh.shape[1]
    rows = np.ascontiguousarray(yh.transpose(1, 0, 2, 3)).reshape(d_out, H, WE)
    return _unpack_bits(rows)


def _np_dilate(vol: np.ndarray, ker: np.ndarray) -> np.ndarray:
    """Generic numpy fallback: conv3d(pad=1) > 0 for an arbitrary 3x3x3
    kernel (matches the reference exactly, including negative weights)."""
    b, ch, dd, hh, ww = vol.shape
    pad = np.pad(vol, ((0, 0), (0, 0), (1, 1), (1, 1), (1, 1)))
    kv = ker.reshape(3, 3, 3).astype(np.float64)
    s = np.zeros(vol.shape, np.float64)
    for i in range(3):
        for j in range(3):
            for k in range(3):
                if kv[i, j, k] != 0.0:
                    s += kv[i, j, k] * pad[:, :, i : i + dd, j : j + hh, k : k + ww]
    return (s > 0).astype(vol.dtype)


def kernel(binary_volume=None, kernel=None, **_unused):
    global _NC_CACHE, LAST_RESULTS
    vol = np.ascontiguousarray(np.asarray(binary_volume), dtype=np.float32)
    ker = np.asarray(kernel, dtype=np.float32)
    kv = ker.reshape(3, 3, 3)
    if (
        vol.shape != (B, 1, D_TOT, H, W)
        or not np.array_equal(kv != 0, _STAR)
        or not (kv[_STAR] > 0).all()
        or not ((vol == 0.0) | (vol == 1.0)).all()
    ):
        return _np_dilate(vol, ker).astype(np.asarray(binary_volume).dtype)

    from concourse.bass_utils import run_bass_kernel_spmd

    xr = vol.reshape(B, D_TOT, H, W)
    in_maps = []
    for core in range(N_CORES):
        b, s = divmod(core, D_SHARDS)
        d0 = s * D_OUT
        slab = np.zeros((D_IN, H, W), np.float32)
        j_lo = 0 if d0 > 0 else 1                      # slab j <-> global d0-1+j
        j_hi = D_IN if d0 + D_OUT < D_TOT else D_IN - 1
        slab[j_lo:j_hi] = xr[b, d0 - 1 + j_lo : d0 - 1 + j_hi]
        in_maps.append(host_inputs(slab))

    if _NC_CACHE is None:
        _NC_CACHE = build_nc()
    res = run_bass_kernel_spmd(_NC_CACHE, in_maps, list(range(N_CORES)), **RUN_KWARGS)
    LAST_RESULTS = res

    full = np.empty((B, 1, D_TOT, H, W), np.float32)
    for core in range(N_CORES):
        b, s = divmod(core, D_SHARDS)
        full[b, 0, s * D_OUT : (s + 1) * D_OUT] = out_to_slab(
            res.results[core]["y"]
        )
    return full
